# revision 16
# baseline (speedup 1.0000x reference)
"""BiLSTM-CRF loss kernel for Trainium2 (8 NeuronCores, data-parallel over batch).

Self-contained: hardcodes B=256, T=512, K=128, START=126, STOP=127.
kernel(**inputs) takes the full inputs and returns (loss, path_score, pred)
matching reference.reference().
"""
import numpy as np
import concourse.bass as bass
import concourse.bacc as bacc
import concourse.tile as tile
from concourse import mybir, bass_isa
from concourse import bass_utils

B, T, K = 256, 512, 128
NCORES = 8
BL = B // NCORES          # 32 sentences per core
START, STOP = 126, 127
NEG = -10000.0
F32 = mybir.dt.float32
I32 = mybir.dt.int32
AX = mybir.AxisListType
OP = mybir.AluOpType
ACT = mybir.ActivationFunctionType
RENORM = 64               # CRF renormalization period
LOG2_8 = 8 * float(np.log(2.0))   # per-step 2^-8 scaling folded into exp(feat)


def _build(T=T, RENORM=RENORM):
    nc = bacc.Bacc("TRN2")

    feats = nc.dram_tensor("feats", [BL, T, K], F32, kind="ExternalInput")
    tags = nc.dram_tensor("tags", [BL, T], I32, kind="ExternalInput")
    trans = nc.dram_tensor("transitions", [K, K], F32, kind="ExternalInput")

    o_logz = nc.dram_tensor("o_logz", [1, BL], F32, kind="ExternalOutput")
    o_emit = nc.dram_tensor("o_emit", [BL, 1], F32, kind="ExternalOutput")
    o_trsc = nc.dram_tensor("o_trsc", [1, BL], F32, kind="ExternalOutput")
    o_stop = nc.dram_tensor("o_stop", [BL, 1], F32, kind="ExternalOutput")
    o_path = nc.dram_tensor("o_path", [1, BL], F32, kind="ExternalOutput")
    o_pred = nc.dram_tensor("o_pred", [T, BL], I32, kind="ExternalOutput")

    # carry alphas for the backtrace: alphas[t] = viterbi carry before feat_t,
    # layout [from, b]. t = 1..512 are written (t=0 is the known init).
    alphas = nc.dram_tensor("alphas_scratch", [T + 1, K, BL], F32)

    with tile.TileContext(nc) as tc, \
         tc.tile_pool(name="big", bufs=1) as big, \
         tc.tile_pool(name="const", bufs=1) as const, \
         tc.tile_pool(name="loop", bufs=2) as loop, \
         tc.tile_pool(name="crf", bufs=3) as crf, \
         tc.tile_pool(name="bwd", bufs=2) as bwd, \
         tc.tile_pool(name="pf", bufs=2) as pf, \
         tc.tile_pool(name="ps", bufs=2, space="PSUM") as ps, \
         tc.tile_pool(name="ps1", bufs=2, space="PSUM") as ps1, \
         tc.tile_pool(name="ps2", bufs=1, space="PSUM") as ps2:

        # ---------------- phase 0: constants and input re-layouts ----------------
        # iota/identity helpers
        iota_col_i = const.tile([K, 1], I32)
        nc.gpsimd.iota(iota_col_i[:], pattern=[[0, 1]], base=0, channel_multiplier=1)
        iota_col = const.tile([K, 1], F32)
        nc.vector.tensor_copy(iota_col[:], iota_col_i[:])
        negi_col = const.tile([K, 1], F32)   # 128 - p
        nc.vector.tensor_scalar(out=negi_col[:], in0=iota_col[:], scalar1=-1.0,
                                scalar2=float(K), op0=OP.mult, op1=OP.add)
        iota_row_i = const.tile([K, K], I32)  # value j in every partition
        nc.gpsimd.iota(iota_row_i[:], pattern=[[1, K]], base=0, channel_multiplier=0)
        iota_row = const.tile([K, K], F32)
        nc.vector.tensor_copy(iota_row[:], iota_row_i[:])
        ident = const.tile([K, K], F32)      # identity for PE transposes
        nc.vector.tensor_tensor(out=ident[:], in0=iota_col[:].to_broadcast((K, K)),
                                in1=iota_row[:], op=OP.is_equal)

        # transitions: natural [to, from], transposed [from, to], exp variants
        tr_sb = const.tile([K, K], F32)
        nc.sync.dma_start(tr_sb[:], trans[:])
        trT_ps = ps1.tile([K, K], F32, tag="scr")
        nc.tensor.matmul(trT_ps[:], tr_sb[:], ident[:], is_transpose=True,
                         start=True, stop=True)
        trT = const.tile([K, K], F32)        # trT[from, to]
        nc.scalar.copy(trT[:], trT_ps[:])
        # E_T[from, to] = exp(clamp(trans[to,from], -200))  (lhsT for CRF matmul)
        trT_cl = const.tile([K, K], F32)
        nc.vector.tensor_scalar(out=trT_cl[:], in0=trT[:], scalar1=-200.0,
                                scalar2=None, op0=OP.max)
        e_t = const.tile([K, K], F32)
        nc.scalar.activation(e_t[:], trT_cl[:], ACT.Exp)
        # w_stop[to, 1] = exp(clamp(trans[STOP, to])): final logZ weights
        stopcol = const.tile([K, 1], F32)
        nc.vector.tensor_scalar(out=stopcol[:], in0=trT[:, STOP:STOP + 1],
                                scalar1=-200.0, scalar2=None, op0=OP.max)
        w_stop = const.tile([K, 1], F32)
        nc.scalar.activation(w_stop[:], stopcol[:], ACT.Exp)

        # trans_rep4[p=(k,b), to32, from] = trans[32k+to32, from]
        tr_rep = big.tile([K, 32, K], F32)
        for k in range(4):
            src = bass.AP(tensor=trans, offset=32 * k * K,
                          ap=[[0, 32], [K, 32], [1, K]])
            nc.sync.dma_start(tr_rep[32 * k:32 * (k + 1), :, :], src)

        # featkb[p=(k,b), t, to32] = feats[b, t, 32k+to32], chunked over t for prefetch
        featkb = big.tile([K, T, 32], F32)
        TCH = min(128, T)
        NCH = T // TCH
        for k in range(4):
            for c in range(NCH):
                src = bass.AP(tensor=feats, offset=32 * k + c * TCH * K,
                              ap=[[T * K, 32], [K, TCH], [1, 32]])
                nc.sync.dma_start(featkb[32 * k:32 * (k + 1), c * TCH:(c + 1) * TCH, :], src)

        # XT[to, (b, t)] = feats[b, t, to] via PE transposes of [t128, k128] chunks
        xt = big.tile([K, BL, T], F32)
        for c in range(NCH):
            for b in range(BL):
                fch = pf.tile([TCH, K], F32, tag="fch")
                nc.sync.dma_start(fch[:], feats[b, c * TCH:(c + 1) * TCH, :])
                tp = ps1.tile([K, TCH], F32, tag="scr")
                nc.tensor.matmul(tp[:], fch[:], ident[0:TCH, 0:TCH], is_transpose=True,
                                 start=True, stop=True)
                nc.scalar.copy(xt[:, b, c * TCH:(c + 1) * TCH], tp[:])

        # tags: [b, t] int32, f32 copy, prev-tags (free-dim shift), and
        # time-transposed variants tags_Tall/tags_pTall [tmod, chunk, b]
        tags_sb = const.tile([BL, T], I32)
        nc.sync.dma_start(tags_sb[:], tags[:])
        tags_f = const.tile([BL, T], F32)
        nc.vector.tensor_copy(tags_f[:], tags_sb[:])
        tags_pf = const.tile([BL, T], F32)
        nc.vector.memset(tags_pf[:, 0:1], float(START))
        nc.vector.tensor_copy(tags_pf[:, 1:T], tags_f[:, 0:T - 1])
        tags_Tall = const.tile([TCH, NCH, BL], F32)
        tags_pTall = const.tile([TCH, NCH, BL], F32)
        for c in range(NCH):
            for src_t, dst_t in ((tags_f, tags_Tall), (tags_pf, tags_pTall)):
                tps = ps1.tile([TCH, BL], F32, tag="scr")
                nc.tensor.matmul(tps[:], src_t[:, c * TCH:(c + 1) * TCH],
                                 ident[0:BL, 0:BL], is_transpose=True,
                                 start=True, stop=True)
                nc.scalar.copy(dst_t[:, c, :], tps[:])

        # ---------------- phase 1: main scan over t ----------------
        # viterbi carry, replicated: alphaT_rep[p, f] = alpha[b=p%32 -> f] (rows repeat every 32)
        alphaT = loop.tile([K, K], F32, tag="alphaT")
        nc.vector.memset(alphaT[0:32, :], NEG)
        nc.vector.memset(alphaT[0:32, START:START + 1], 0.0)
        for k in range(1, 4):
            nc.vector.tensor_copy(alphaT[32 * k:32 * (k + 1), :], alphaT[0:32, :])
        # CRF scaled-linear state ea[from, b]; running log-correction Macc[1, b]
        ea = crf.tile([K, BL], F32, tag="ea")
        nc.vector.tensor_scalar(out=ea[:], in0=iota_col[:].to_broadcast((K, BL)),
                                scalar1=float(START), scalar2=None, op0=OP.is_equal)
        macc = const.tile([1, BL], F32)
        nc.vector.memset(macc[:], 0.0)

        # store initial viterbi carry (t=0): [from, b] = NEG except START row
        a0 = const.tile([K, BL], F32)
        nc.vector.tensor_scalar(out=a0[:], in0=ea[:], scalar1=-NEG, scalar2=NEG,
                                op0=OP.mult, op1=OP.add)
        nc.sync.dma_start(alphas[0, :, :], a0[:])

        # constant bias tiles for activations
        bias_l8 = const.tile([K, 1], F32)
        nc.vector.memset(bias_l8[:], -LOG2_8)
        bias_fin = const.tile([1, 1], F32)
        nc.vector.memset(bias_fin[:], T * LOG2_8)

        scores = big.tile([K, 32, K], F32)   # bufs=1: strictly serial use
        for t in range(T):
            # V1: scores[p, to32, f] = trans_rep4 + alphaT_rep (bcast over to32)
            a_b = bass.AP(tensor=alphaT.tensor, offset=alphaT.offset,
                          ap=[alphaT.ap[0], [0, 32], alphaT.ap[1]])
            nc.vector.tensor_tensor(out=scores[:], in0=tr_rep[:], in1=a_b, op=OP.add)
            # V2: q[p, to32] = max_f scores
            q = loop.tile([K, 32], F32, tag="q")
            nc.vector.tensor_reduce(q[:], scores[:], axis=AX.X, op=OP.max)
            # V3: add emissions (same-partition TT), V4: gather quadrants into
            # new alphaT rows 0:32 (verified cross-partition-offset copies)
            q2 = loop.tile([K, 32], F32, tag="q2")
            nc.vector.tensor_tensor(out=q2[:], in0=q[:], in1=featkb[:, t, :], op=OP.add)
            alphaT = loop.tile([K, K], F32, tag="alphaT")
            for k in range(4):
                nc.vector.tensor_copy(alphaT[0:32, 32 * k:32 * (k + 1)],
                                      q2[32 * k:32 * (k + 1), :])
            # V5: replicate rows 0:32 into quadrants 1..3
            for k in range(1, 4):
                nc.vector.tensor_copy(alphaT[32 * k:32 * (k + 1), :], alphaT[0:32, :])
            # V6: store carry_{t+1} as [from, b] via PE transpose
            afb_ps = ps.tile([K, 32], F32, tag="afb")
            nc.tensor.matmul(afb_ps[:], alphaT[0:32, :], ident[0:32, 0:32],
                             is_transpose=True, start=True, stop=True)
            afb = loop.tile([K, 32], F32, tag="afb_sb")
            nc.scalar.copy(afb[:], afb_ps[:])
            nc.sync.dma_start(alphas[t + 1, :, :], afb[:])

            # CRF: S = E_T.T @ ea ; ea' = S * exp(feat_t - 8*ln2)
            s_ps = ps.tile([K, BL], F32, tag="s")
            nc.tensor.matmul(s_ps[:], e_t[:], ea[:], start=True, stop=True)
            ef = crf.tile([K, BL], F32, tag="ef")
            xslice = bass.AP(tensor=xt.tensor, offset=xt.offset + t,
                             ap=[xt.ap[0], [T, BL]])
            nc.scalar.activation(ef[:], xslice, ACT.Exp, bias=bias_l8[:], scale=1.0)
            ea = crf.tile([K, BL], F32, tag="ea")
            nc.vector.tensor_tensor(out=ea[:], in0=s_ps[:], in1=ef[:], op=OP.mult)

            if (t + 1) % RENORM == 0:
                cmax = crf.tile([K, BL], F32, tag="cmax")
                nc.gpsimd.partition_all_reduce(cmax[:], ea[:], channels=K,
                                               reduce_op=bass_isa.ReduceOp.max)
                crec = crf.tile([K, BL], F32, tag="crec")
                nc.vector.reciprocal(crec[:], cmax[:])
                ea2 = crf.tile([K, BL], F32, tag="ea")
                nc.vector.tensor_tensor(out=ea2[:], in0=ea[:], in1=crec[:], op=OP.mult)
                ea = ea2
                lnc = crf.tile([1, BL], F32, tag="lnc")
                nc.scalar.activation(lnc[:], cmax[0:1, :], ACT.Ln)
                nc.vector.tensor_tensor(out=macc[:], in0=macc[:], in1=lnc[:], op=OP.add)

        # ---------------- phase 2: CRF finalize ----------------
        z_ps = ps1.tile([1, BL], F32, tag="scr")
        nc.tensor.matmul(z_ps[:], w_stop[:], ea[:], start=True, stop=True)
        lz = const.tile([1, BL], F32)
        nc.scalar.activation(lz[:], z_ps[:], ACT.Ln)
        # logZ = ln(z) + Macc + T*8*ln2
        nc.vector.tensor_tensor(out=lz[:], in0=lz[:], in1=macc[:], op=OP.add)
        lzf = const.tile([1, BL], F32)
        nc.scalar.activation(lzf[:], lz[:], ACT.Identity, bias=bias_fin[:], scale=1.0)
        nc.sync.dma_start(o_logz[:], lzf[:])

        # ---------------- phase 2b: gold score pieces ----------------
        # emit[b] = sum_t feats[b,t,tags[b,t]] via eq-mask over featkb
        # tagskb[p=(k,b), t] = tags[b, t] (4 bcast-style dma loads from dram)
        tagskb = const.tile([K, T], I32)
        for k in range(4):
            src = bass.AP(tensor=tags, offset=0, ap=[[T, 32], [1, T]])
            nc.sync.dma_start(tagskb[32 * k:32 * (k + 1), :], src)
        tagskb_f = const.tile([K, T], F32)
        nc.vector.tensor_copy(tagskb_f[:], tagskb[:])
        # iotaq[p, to32] = 32*(p//32) + to32
        iotaq = const.tile([K, 32], F32)
        for k in range(4):
            ioq_i = pf.tile([32, 32], I32, tag="ioq")
            nc.gpsimd.iota(ioq_i[:], pattern=[[1, 32]], base=32 * k, channel_multiplier=0)
            nc.vector.tensor_copy(iotaq[32 * k:32 * (k + 1), :], ioq_i[:])
        emred4 = const.tile([K, NCH], F32)
        for c in range(NCH):
            t0c, t1c = c * TCH, (c + 1) * TCH
            eqc = big.tile([K, TCH, 32], F32, tag="scores")
            tkb_b = bass.AP(tensor=tagskb_f.tensor, offset=tagskb_f.offset + t0c,
                            ap=[tagskb_f.ap[0], [1, TCH], [0, 32]])
            ioq_b = bass.AP(tensor=iotaq.tensor, offset=iotaq.offset,
                            ap=[iotaq.ap[0], [0, TCH], iotaq.ap[1]])
            nc.vector.tensor_tensor(out=eqc[:], in0=tkb_b, in1=ioq_b, op=OP.is_equal)
            nc.vector.tensor_tensor(out=eqc[:], in0=eqc[:],
                                    in1=featkb[:, t0c:t1c, :], op=OP.mult)
            nc.vector.tensor_reduce(emred4[:, c:c + 1], eqc[:], axis=AX.XY, op=OP.add)
        emred = const.tile([K, 1], F32)
        nc.vector.tensor_reduce(emred[:], emred4[:], axis=AX.X, op=OP.add)
        # sum the 4 quadrants via copies into columns, then a free-dim reduce
        emq = const.tile([32, 4], F32)
        for k in range(4):
            nc.vector.tensor_copy(emq[:, k:k + 1], emred[32 * k:32 * (k + 1), :])
        emred3 = const.tile([32, 1], F32)
        nc.vector.tensor_reduce(emred3[:], emq[:], axis=AX.X, op=OP.add)
        nc.sync.dma_start(o_emit[:], emred3[:])

        # trans_sc[b] = sum_t trans[c_t, p_t], pairs (c_t, p_t): c=tags[t], p=tags[t-1], p_0=START
        # via count-matrix C_b = sum_chunks OHc_chunk.T-ish products on PE
        trsc_asm = const.tile([K, BL], F32)
        for b in range(BL):
            c_ps = ps2.tile([K, K], F32, tag="cmat")
            for c in range(NCH):
                # OHc[t, i] = (tags[t] == i); OHp[t, j] = (tags[t-1] == j)
                ohc = pf.tile([TCH, K], F32, tag="ohc")
                nc.vector.tensor_tensor(
                    out=ohc[:], in0=tags_Tall[:, c, b:b + 1].to_broadcast((TCH, K)),
                    in1=iota_row[0:TCH, :], op=OP.is_equal)
                ohp = pf.tile([TCH, K], F32, tag="ohp")
                nc.vector.tensor_tensor(
                    out=ohp[:], in0=tags_pTall[:, c, b:b + 1].to_broadcast((TCH, K)),
                    in1=iota_row[0:TCH, :], op=OP.is_equal)
                nc.tensor.matmul(c_ps[:], ohc[:], ohp[:], start=(c == 0), stop=(c == NCH - 1))
            cts = pf.tile([K, K], F32, tag="cts")
            nc.vector.tensor_tensor(out=cts[:], in0=c_ps[:], in1=tr_sb[:], op=OP.mult)
            ctr = pf.tile([K, 1], F32, tag="ctr")
            nc.vector.tensor_reduce(ctr[:], cts[:], axis=AX.X, op=OP.add)
            nc.gpsimd.partition_all_reduce(trsc_asm[:, b:b + 1], ctr[:], channels=K,
                                           reduce_op=bass_isa.ReduceOp.add)
        nc.sync.dma_start(o_trsc[:], trsc_asm[0:1, :])

        # stop_sc[b] = trans[STOP, tags[b, T-1]] via one-hot matmul
        ohlast = const.tile([K, BL], F32)
        lastrep = const.tile([K, BL], F32)
        lastcol = const.tile([1, BL], F32)
        # tags_f[:, T-1] is [BL, 1] on partitions 0..31; transpose via PE
        lt_ps = ps1.tile([1, BL], F32, tag="scr")
        nc.tensor.matmul(lt_ps[:], tags_f[:, T - 1:T], ident[0:BL, 0:BL],
                         is_transpose=True, start=True, stop=True)
        nc.scalar.copy(lastcol[:], lt_ps[:])
        nc.gpsimd.partition_broadcast(lastrep[:], lastcol[:])
        nc.vector.tensor_tensor(out=ohlast[:], in0=iota_col[:].to_broadcast((K, BL)),
                                in1=lastrep[:], op=OP.is_equal)
        st_ps = ps1.tile([BL, 1], F32, tag="scr")
        nc.tensor.matmul(st_ps[:], ohlast[:], trT[:, STOP:STOP + 1], start=True, stop=True)
        st_sb = const.tile([BL, 1], F32)
        nc.scalar.copy(st_sb[:], st_ps[:])
        nc.sync.dma_start(o_stop[:], st_sb[:])

        # ---------------- phase 3: viterbi backtrace ----------------
        # init: s[to, b] = alpha_T512[to, b] + trans[STOP, to]
        a_last = bwd.tile([K, BL], F32, tag="afb_load")
        nc.sync.dma_start(a_last[:], alphas[T, :, :])
        s0 = bwd.tile([K, BL], F32, tag="s")
        nc.vector.tensor_scalar(out=s0[:], in0=a_last[:], scalar1=stopcol[:],
                                scalar2=None, op0=OP.add)

        def argmax_rep(s_tile, tagname):
            """argmax over partitions of s_tile[K, BL]; returns (max_rep, tag_rep)."""
            smax = bwd.tile([K, BL], F32, tag="smax")
            nc.gpsimd.partition_all_reduce(smax[:], s_tile[:], channels=K,
                                           reduce_op=bass_isa.ReduceOp.max)
            eq = bwd.tile([K, BL], F32, tag="eq")
            nc.vector.tensor_tensor(out=eq[:], in0=s_tile[:], in1=smax[:], op=OP.is_equal)
            val = bwd.tile([K, BL], F32, tag="val")
            nc.vector.tensor_scalar(out=val[:], in0=eq[:], scalar1=negi_col[:],
                                    scalar2=None, op0=OP.mult)
            vmax = bwd.tile([K, BL], F32, tag="vmax")
            nc.gpsimd.partition_all_reduce(vmax[:], val[:], channels=K,
                                           reduce_op=bass_isa.ReduceOp.max)
            tag_rep = bwd.tile([K, BL], F32, tag=tagname)
            nc.vector.tensor_scalar(out=tag_rep[:], in0=vmax[:], scalar1=-1.0,
                                    scalar2=float(K), op0=OP.mult, op1=OP.add)
            return smax, tag_rep

        psc, tag_rep = argmax_rep(s0, "tag_rep")
        nc.sync.dma_start(o_path[:], psc[0:1, :])
        prow = bwd.tile([1, BL], I32, tag="prow")
        nc.vector.tensor_copy(prow[:], tag_rep[0:1, :])
        nc.sync.dma_start(o_pred[T - 1:T, :], prow[:])

        for t in range(T - 1, 0, -1):
            onehot = bwd.tile([K, BL], F32, tag="onehot")
            nc.vector.tensor_tensor(out=onehot[:], in0=iota_col[:].to_broadcast((K, BL)),
                                    in1=tag_rep[:], op=OP.is_equal)
            trow_ps = ps.tile([K, BL], F32, tag="s")
            nc.tensor.matmul(trow_ps[:], tr_sb[:], onehot[:], start=True, stop=True)
            a_t = bwd.tile([K, BL], F32, tag="afb_load")
            nc.sync.dma_start(a_t[:], alphas[t, :, :])
            s_t = bwd.tile([K, BL], F32, tag="s")
            nc.vector.tensor_tensor(out=s_t[:], in0=a_t[:], in1=trow_ps[:], op=OP.add)
            _, tag_rep = argmax_rep(s_t, "tag_rep")
            prow = bwd.tile([1, BL], I32, tag="prow")
            nc.vector.tensor_copy(prow[:], tag_rep[0:1, :])
            nc.sync.dma_start(o_pred[t - 1:t, :], prow[:])

    nc.compile()
    return nc


_NC_CACHE = None


def _get_nc():
    global _NC_CACHE
    if _NC_CACHE is None:
        _NC_CACHE = _build()
    return _NC_CACHE


def kernel(feats, tags, transitions):
    feats = np.ascontiguousarray(np.asarray(feats), dtype=np.float32)
    tags_i = np.ascontiguousarray(np.asarray(tags)).astype(np.int32)
    trans = np.ascontiguousarray(np.asarray(transitions), dtype=np.float32)

    nc = _get_nc()
    in_maps = []
    for c in range(NCORES):
        sl = slice(c * BL, (c + 1) * BL)
        in_maps.append({
            "feats": feats[sl],
            "tags": tags_i[sl],
            "transitions": trans,
        })
    res = bass_utils.run_bass_kernel_spmd(nc, in_maps, core_ids=list(range(NCORES)))

    logz = np.concatenate([r["o_logz"][0] for r in res.results])          # [B]
    emit = np.concatenate([r["o_emit"][:, 0] for r in res.results])       # [B]
    trsc = np.concatenate([r["o_trsc"][0] for r in res.results])          # [B]
    stop = np.concatenate([r["o_stop"][:, 0] for r in res.results])       # [B]
    path_score = np.concatenate([r["o_path"][0] for r in res.results])    # [B]
    pred = np.concatenate([r["o_pred"].T for r in res.results], axis=0)   # [B, T]

    gold = trsc + emit + stop
    loss = np.float32(np.sum(logz.astype(np.float64) - gold.astype(np.float64))
                      / (B * T))
    return loss, path_score.astype(np.float32), pred.astype(np.int32)
